# revision 53
# baseline (speedup 1.0000x reference)
"""Multi-head attention (b=2, sq=skv=2048, dim=1024, 16 heads x 64) on 8 TRN2
NeuronCores.

Sharding: 2 heads per core (head-parallel across batch*heads), with the
matching tensor-parallel column slice of W_qkv and row slice of W_out.  Each
core computes a partial output projection over its 128 head-dims; the
all-reduce of the 8 partials (+ bias) happens on the host during unshard.

Per-core kernel (bf16 compute, fp32 PSUM accumulation):
  phase 1: qT/kT/vT = W.T @ x.T   ([128 = 2 heads x 64 dims, tokens]); v is
           additionally PE-transposed to natural [token, dim] layout with a
           ones column appended (denominator trick).
  phase 2: per (batch, q-tile, k-tile): scoresT for both heads ([k-tokens, q])
           as a concurrent row-group pair in one 2-bank PSUM group; one exp
           op per step over the group, split across engines by k-tile (DVE
           Schraudolph on 7 of 16, exact ACT exp on the rest; scale 1/8
           fused, no max subtraction -- scores range +-10); PV matmuls
           accumulate [v | 1].T @ expT over the 16 k-tiles giving
           unnormalized outT plus the per-head softmax denominators in row
           64.  The accumulators drain to SBUF immediately (dims into usb's
           two partition bands, denominators into denb); normalization is
           full-width off the critical path: one K=65 selector matmul
           broadcasts both heads' denominators, then a single 128-partition
           reciprocal and multiply per q-tile.
  phase 3: partial out = outT.T @ W_out_rows -> bf16 [tokens, 1024].

Emission is orchestrated so the dependency-driven Tile scheduler always has
filler PE work (batch-1 projections, out-projection quarters) inside the
ACT(exp)-bound attention stream, keeping the PE HAM-warm.
"""

import os
import sys

for _p in ("/opt/trn_rl_repo", "/root/.axon_site/_ro/trn_rl_repo"):
    if os.path.isdir(_p) and _p not in sys.path:
        sys.path.append(_p)

import ml_dtypes
import numpy as np

import concourse.bass as bass  # noqa: F401
import concourse.tile as tile
from concourse import bacc, mybir
from concourse.bass_utils import run_bass_kernel_spmd
from concourse.masks import make_identity

B, SQ, SKV, DIM = 2, 2048, 2048, 1024
HEADS, DH = 16, 64
N_CORES = 8
HPC = HEADS // N_CORES  # heads per core = 2
HD = HPC * DH  # 128 head-dim rows per core
TOK = B * SQ  # 4096
KO = DIM // 128  # 8 contraction chunks of 128
SCALE = DH**-0.5

BF16 = mybir.dt.bfloat16
F32 = mybir.dt.float32

PCHUNK = 512  # token chunk in projections (contiguous per-chunk dram layout)
QTILE = 512  # q tile in attention
KTILE = 128  # k tile (scores psum partition dim)
NKT = SKV // KTILE  # 16
NQT = SQ // QTILE  # 4

BF = ml_dtypes.bfloat16
Exp = mybir.ActivationFunctionType.Exp
I16 = mybir.dt.int16
Mult = mybir.AluOpType.mult
Add = mybir.AluOpType.add

# bf16-bitspace Schraudolph exp: bits = rint(s*SCALE*128/ln2 + (127*128 - 5.49))
# bitcast int16->bf16 gives ~exp(s*SCALE) with ~2% rms sawtooth error that is
# common-mode-cancelled by the softmax normalization.  7 of 16 k-tiles run it
# on the DVE (the rest use exact exp on ACT) so each engine carries only part
# of the per-step exp -- the exp latency recycles the score psum bufs, which
# is what gates the score matmul pairs on the PE.
SCH_C1 = float(SCALE * 128.0 / np.log(2.0))
SCH_C2 = 16256.0 - 5.49
DVE_J16 = frozenset((1, 3, 5, 7, 9, 11, 13))


def build():
    nc = bacc.Bacc(
        "TRN2", target_bir_lowering=False, debug=False, num_devices=N_CORES
    )

    NCH = TOK // PCHUNK
    xqt_d = nc.dram_tensor("xqt", [NCH, 128, KO, PCHUNK], BF16, kind="ExternalInput")
    xkvt_d = nc.dram_tensor("xkvt", [NCH, 128, KO, PCHUNK], BF16, kind="ExternalInput")
    wq_d = nc.dram_tensor("wq", [DIM, HD], BF16, kind="ExternalInput")
    wk_d = nc.dram_tensor("wk", [DIM, HD], BF16, kind="ExternalInput")
    wv_d = nc.dram_tensor("wv", [DIM, HD], BF16, kind="ExternalInput")
    wout_d = nc.dram_tensor("wout", [HD, DIM], BF16, kind="ExternalInput")
    out_d = nc.dram_tensor("out", [TOK, DIM], BF16, kind="ExternalOutput")

    xqt = xqt_d.ap()
    xkvt = xkvt_d.ap()

    with tile.TileContext(nc) as tc:
        with (
            tc.tile_pool(name="persist", bufs=1) as persist,
            tc.tile_pool(name="xin", bufs=4) as xin,
            tc.tile_pool(name="exps", bufs=10) as exps,
            tc.tile_pool(name="ost", bufs=3) as ost,
            tc.tile_pool(name="spsum", bufs=2, space="PSUM") as spsum,
            tc.tile_pool(name="accp", bufs=2, space="PSUM") as accp,
            tc.tile_pool(name="miscp", bufs=2, space="PSUM") as miscp,
        ):
            # PE p-state warmup, first thing: memset on the (idle) gpsimd
            # queue, then dummy matmuls so the PE is continuously busy from
            # ~6.3us (right after the framework preamble).  The HAM clock
            # gate holds 2.4GHz only while the PE never idles >~3.4us.
            warm = persist.tile([128, 512], BF16, tag="warm")
            nc.gpsimd.memset(warm[:], 0.0)
            for _ in range(8):
                wps = spsum.tile([128, QTILE], F32, tag="s", name="warmup")
                nc.tensor.matmul(wps[:], warm[:, 0:128], warm[:], start=True, stop=True)

            def filler(n):
                # keep-warm matmuls emitted between DMA-gated projection
                # chains in the startup region and around the tail (their
                # spsum WAR deps are quiet there; never mid-attention)
                for _ in range(n):
                    f = spsum.tile([128, QTILE], F32, tag="s", name="fill")
                    nc.tensor.matmul(
                        f[:], warm[:, 0:128], warm[:], start=True, stop=True
                    )

            def fillera(n):
                # early-attention keep-warm filler on accp (free until the
                # first PV accumulators are allocated at step 6)
                for _ in range(n):
                    f = accp.tile([128, QTILE], F32, tag="acc", name="filla")
                    nc.tensor.matmul(
                        f[:], warm[:, 0:128], warm[:], start=True, stop=True
                    )

            def fillerm(n):
                # drain-region keep-warm filler on the miscp pool (spsum
                # still cycles live score tiles there)
                for _ in range(n):
                    f = miscp.tile([128, 512], F32, tag="m", name="fillm")
                    nc.tensor.matmul(
                        f[:], warm[:, 0:128], warm[:], start=True, stop=True
                    )

            # --- weights / constants.  Startup queue plan (per-queue dynamic
            # DMA sustains ~160GB/s; only sync/gpsimd/scalar can issue): the
            # first k/q chunks are split as quarters across all three
            # queues so attention can start by ~13.5us at full clock. ---
            wk_sb = persist.tile([128, KO, HD], BF16, tag="wk")
            nc.sync.dma_start(wk_sb[:], wk_d.ap().rearrange("(ko p) m -> p ko m", p=128))
            wq_sb = persist.tile([128, KO, HD], BF16, tag="wq")
            nc.scalar.dma_start(wq_sb[:], wq_d.ap().rearrange("(ko p) m -> p ko m", p=128))

            ident = persist.tile([128, 128], BF16, tag="ident")
            make_identity(nc, ident[:])
            ones_f32 = persist.tile([128, DH], F32, tag="ones")
            nc.vector.memset(ones_f32[:], 1.0)
            ones_bf = persist.tile([1, DH], BF16, tag="onesb")
            nc.vector.memset(ones_bf[:], 1.0)
            # 65x128 selector: broadcast head h's denominator (kept on
            # partition h*64 of denb) to outT's partition band h*64..h*64+63
            # in one matmul; rows 1..63 are zero.  65 keeps every engine AP
            # 32-aligned (partition bases 0 and 64).
            sel = persist.tile([65, 128], BF16, tag="sel")
            nc.vector.memset(sel[:], 0.0)
            nc.vector.memset(sel[0:1, 0:DH], 1.0)
            nc.vector.memset(sel[DH : DH + 1, DH : 2 * DH], 1.0)

            qt_sb, kt_sb, vt_sb, vnat, outT, usb, den = {}, {}, {}, {}, {}, {}, {}
            for b in range(B):
                qt_sb[b] = persist.tile([HD, SQ], BF16, tag=f"qt{b}", name=f"qt{b}")
                kt_sb[b] = persist.tile([HD, SKV], BF16, tag=f"kt{b}", name=f"kt{b}")
                vt_sb[b] = persist.tile([HD, SKV], BF16, tag=f"vt{b}", name=f"vt{b}")
                vnat[b] = persist.tile(
                    [128, NKT, HPC, DH + 1], BF16, tag=f"vn{b}", name=f"vn{b}"
                )
                outT[b] = persist.tile([HD, SQ], BF16, tag=f"ot{b}", name=f"ot{b}")
                # unnormalized outT (h0 dims on partitions 0-63, h1 on
                # 64-127) + the per-head denominator rows, per q-tile
                usb[b] = persist.tile(
                    [128, NQT, QTILE], F32, tag=f"us{b}", name=f"us{b}"
                )
                den[b] = persist.tile(
                    [DH + 1, NQT, QTILE], BF16, tag=f"dn{b}", name=f"dn{b}"
                )
                # rows 1..63 are never written but ARE streamed through the
                # K=65 selector matmul (against zero weights) -- zero them
                # so no garbage bit-pattern can decode to NaN/Inf
                nc.vector.memset(den[b][:], 0.0)
                nc.vector.memset(vnat[b][:, :, :, DH], 1.0)

            def _proj(dst, w_sb, xt, tt, scale=None, fill_at=None):
                for sub in range(PCHUNK // 512):
                    _proj_sub(dst, w_sb, xt, tt, sub, scale, fill_at or {})

            KOH = KO // 2

            def load_chunk(x_ap, tok0, tt, engs=None, fine=False):
                """Load a 512-token chunk split by ko across DMA queues: the
                projection's ko-chain starts as soon as the first piece
                lands.  fine=True splits into 4 quarter-tiles round-robined
                over the given queues for the startup-critical chunks, so
                the first matmuls gate on 256KB instead of 512KB and all
                three dynamic queues stream concurrently."""
                engs = engs or (nc.sync, nc.gpsimd)
                ch = x_ap[(tok0 + tt * PCHUNK) // PCHUNK]
                nt = 4 if fine else 2
                w = KO // nt
                tiles = []
                for i in range(nt):
                    xt = xin.tile(
                        [128, w, PCHUNK], BF16, tag=f"xf{i}" if fine else "ab"[i] + "x"
                    )
                    engs[i % len(engs)].dma_start(
                        xt[:], ch[:, i * w : (i + 1) * w, :]
                    )
                    tiles.append(xt)
                return tiles

            def _proj_sub(dst, w_sb, xt, tt, sub, scale=None, fill_at={}):
                ps = miscp.tile([128, 512], F32, tag="m", name="projp")
                w = KO // len(xt)
                for ko in range(KO):
                    if ko in fill_at:
                        # cover this chain's own DMA-quarter waits so the
                        # HAM clock gate stays released during startup
                        filler(fill_at[ko])
                    part = xt[ko // w]
                    nc.tensor.matmul(
                        ps[:],
                        w_sb[:, ko, :],
                        part[:, ko % w, sub * 512 : (sub + 1) * 512],
                        start=(ko == 0),
                        stop=(ko == KO - 1),
                    )
                t0 = tt * PCHUNK + sub * 512
                # ACT does the psum drain: the DVE is loaded with the per-step
                # Schraudolph exps, and these copies gate miscp recycling
                nc.scalar.activation(
                    dst[:, t0 : t0 + 512],
                    ps[:],
                    mybir.ActivationFunctionType.Copy,
                    scale=scale if scale is not None else 1.0,
                )

            def vnat_group(b, jg):
                """PE-transpose k-tiles 4jg..4jg+3 of vT into natural
                layout: one full 128x128 transpose per k-tile covers BOTH
                heads (their 64-dim bands land side by side in the output
                columns), and one strided ACT copy scatters the group into
                vnat's per-head slots -- half the transpose instructions
                and copies of the per-head variant."""
                tps = miscp.tile([128, 4, 128], BF16, tag="m", name="vtp")
                for i in range(4):
                    j = jg * 4 + i
                    nc.tensor.transpose(
                        tps[:, i, :],
                        vt_sb[b][:, j * KTILE : (j + 1) * KTILE],
                        ident[:],
                    )
                # gpsimd cannot read PSUM; ACT drains (DVE carries the
                # per-step Schraudolph exps)
                nc.scalar.copy(
                    vnat[b][:, jg * 4 : (jg + 1) * 4, :, 0:DH],
                    tps[:].rearrange("p i (h d) -> p i h d", h=HPC),
                )

            def norm_flush(b, qt):
                """Normalize q-tile qt of usb[b] (both heads) into outT[b].

                Full-width: both heads' denominators broadcast in ONE K=2
                PE matmul against the 2x128 selector (rows 0-63 get head
                0's denominator, 64-127 head 1's), then a single
                128-partition reciprocal and multiply.  Half-width (64
                partition) DVE ops pay the same ~0.3us fixed overhead for
                half the work, so processing the head pair together nearly
                halves the engine cost.
                """
                bcd = miscp.tile([128, QTILE], F32, tag="m", name="bcd")
                nc.tensor.matmul(
                    bcd[:], sel[:], den[b][:, qt, :], start=True, stop=True
                )
                rbc = ost.tile([128, QTILE], F32, tag="rb", name="rbc")
                nc.vector.reciprocal_approx_fast(rbc[:], bcd[:])
                nc.vector.tensor_mul(
                    outT[b][:, qt * QTILE : (qt + 1) * QTILE],
                    usb[b][:, qt, :],
                    rbc[:],
                )

            LOOKAHEAD = 6

            def attention(b, hooks, pre=None):
                """Flat software-pipelined attention over all (qt, j) steps.

                Scores for step t+2 are emitted before PV of step t, so the
                PE always has score matmuls queued ahead of the exp/PV chain
                and q-tile boundaries pipeline seamlessly.  hooks is a dict
                keyed (qt, j) of emission callables fired right after that
                step's PV matmuls; pre is keyed by flat step t and fires
                BEFORE that step's scores (for work the scores depend on,
                e.g. the batch's own later qkv chunks).
                """
                NT = NQT * NKT
                sps, exs, accs = {}, {}, {}
                pre = pre or {}

                def emit_scores(t):
                    qt, j = divmod(t, NKT)
                    q_sl = slice(qt * QTILE, (qt + 1) * QTILE)
                    k_sl = slice(j * KTILE, (j + 1) * KTILE)
                    sp = spsum.tile([128, HPC, QTILE], F32, tag="s", name="sp")
                    sps[t] = sp
                    for h in range(HPC):
                        h_sl = slice(h * DH, (h + 1) * DH)
                        nc.tensor.matmul(
                            sp[:, h, :],
                            kt_sb[b][h_sl, k_sl],
                            qt_sb[b][h_sl, q_sl],
                            start=True,
                            stop=True,
                        )

                def emit_exp(t):
                    # One exp instruction per step over the whole
                    # [128,2,512] pair (the ~0.3us fixed overhead per
                    # ACT/DVE op makes per-head instructions a net loss),
                    # engines split by k-tile: DVE Schraudolph (bf16
                    # bitspace) on 7 of 16 j's, exact ACT exp on the rest.
                    # Emitted at PV time: the engines then never race ahead
                    # of the PE and head-of-line block on unfinished scores
                    # (measured ~2us ACT stalls when emitted earlier).
                    sp = sps.pop(t)
                    # kt is pre-scaled by SCH_C1 at its projection drain,
                    # so the scores psum is already in Schraudolph bit-space:
                    # the DVE op is a 1-op add (a 2-op tensor_scalar is ~12%
                    # slower on the loaded DVE), and ACT steps compensate
                    # exactly with scale = ln2/128
                    if (t % NKT) in DVE_J16:
                        exi = exps.tile([128, HPC, QTILE], I16, tag="ei", name="exi")
                        nc.vector.tensor_scalar(
                            out=exi[:],
                            in0=sp[:],
                            scalar1=SCH_C2,
                            scalar2=None,
                            op0=Add,
                        )
                        exs[t] = exi[:].bitcast(BF16)
                    else:
                        exf = exps.tile([128, HPC, QTILE], BF16, tag="e", name="ex")
                        nc.scalar.activation(
                            exf[:], sp[:], Exp, scale=float(SCALE / SCH_C1)
                        )
                        exs[t] = exf[:]

                def emit_tail(t):
                    qt, j = divmod(t, NKT)
                    ex = exs.pop(t)
                    exh = (ex[:, 0, :], ex[:, 1, :])
                    if j == 0:
                        accs[qt] = {}
                    for h in range(HPC):
                        if j == 0:
                            # lazy per-head alloc: h's buffer only gates on
                            # the drain of the PREVIOUS qt's same-head
                            # accumulator, which runs in parallel on two
                            # engines (h0 on ACT, h1 on DVE)
                            accs[qt][h] = accp.tile(
                                [128, QTILE], F32, tag="acc", name="acc"
                            )
                        nc.tensor.matmul(
                            accs[qt][h][0 : DH + 1, :],
                            vnat[b][:, j, h, :],
                            exh[h],
                            start=(j == 0),
                            stop=(j == NKT - 1),
                        )
                    if j == NKT - 1:
                        # free the PSUM accumulators (dims into usb's two
                        # partition bands, denominator rows into den);
                        # normalization comes later in norm_flush
                        nc.scalar.copy(usb[b][0:DH, qt, :], accs[qt][0][0:DH, :])
                        nc.scalar.copy(
                            den[b][0:1, qt, :], accs[qt][0][DH : DH + 1, :]
                        )
                        nc.vector.tensor_copy(
                            usb[b][DH : 2 * DH, qt, :], accs[qt][1][0:DH, :]
                        )
                        nc.vector.tensor_copy(
                            den[b][DH : DH + 1, qt, :],
                            accs[qt][1][DH : DH + 1, :],
                        )
                        del accs[qt]
                    for fn in hooks.get((qt, j), []):
                        fn()

                for t in range(NT + LOOKAHEAD):
                    if t < NT:
                        for fn in pre.get(t, []):
                            fn()
                        emit_scores(t)
                    if t >= LOOKAHEAD:
                        emit_exp(t - LOOKAHEAD)
                        emit_tail(t - LOOKAHEAD)

            def outproj(b, tt0, tt1, split_copy=False, pools=None):
                for tt in range(tt0, tt1):
                    t_sl = slice(tt * 128, (tt + 1) * 128)
                    ob = ost.tile([128, 2, 512], BF16, tag="o")
                    for nt in range(DIM // 512):
                        if pools is None:
                            ps = miscp.tile([128, 512], F32, tag="m", name="projo")
                        else:
                            # tail: spread across free PSUM pools so the
                            # matmuls aren't gated by copy-recycle latency
                            pool, ptag = pools[(tt * 2 + nt) % len(pools)]
                            ps = pool.tile([128, 512], F32, tag=ptag, name="projo")
                        nc.tensor.matmul(
                            ps[:],
                            outT[b][:, t_sl],
                            wout_sb[:, nt * 512 : (nt + 1) * 512],
                            start=True,
                            stop=True,
                        )
                        if split_copy and nt % 2 == 0:
                            nc.scalar.copy(ob[:, nt, :], ps[:])
                        else:
                            nc.vector.tensor_copy(ob[:, nt, :], ps[:])
                        if pools is not None:
                            # tail: ship each half as soon as its copy lands,
                            # alternating queues, so the final DMA drain
                            # overlaps the remaining matmuls
                            eng = nc.gpsimd if (tt * 2 + nt) % 2 else nc.sync
                            eng.dma_start(
                                out_d.ap()[
                                    b * SQ + tt * 128 : b * SQ + (tt + 1) * 128,
                                    nt * 512 : (nt + 1) * 512,
                                ],
                                ob[:, nt, :],
                            )
                    if pools is None:
                        (nc.gpsimd if tt % 2 else nc.sync).dma_start(
                            out_d.ap()[
                                b * SQ + tt * 128 : b * SQ + (tt + 1) * 128, :
                            ].rearrange("t (n c) -> t n c", n=2),
                            ob[:],
                        )

            def qkv_pieces(b):
                """Projection emission steps, 512-token chunks.  K chunks
                first (scores consume them progressively); each x_kv chunk
                is loaded once for both K and V; Q tiles beyond the first
                q-tile come last."""
                xts = {}

                def kv_load_k(tt):
                    xts[tt] = load_chunk(xkvt, b * SKV, tt)
                    _proj(kt_sb[b], wk_sb, xts[tt], tt, scale=SCH_C1)

                def v_part(tt):
                    _proj(vt_sb[b], wv_sb, xts.pop(tt), tt)
                    vnat_group(b, tt)

                yield lambda: kv_load_k(0)
                yield lambda: proj_chunk(qt_sb[b], wq_sb, xqt, b * SQ, 0)
                yield lambda: v_part(0)
                for tt in range(1, SQ // PCHUNK):
                    yield lambda tt=tt: kv_load_k(tt)
                    yield lambda tt=tt: v_part(tt)
                for tt in range(1, SQ // PCHUNK):
                    yield lambda tt=tt: proj_chunk(qt_sb[b], wq_sb, xqt, b * SQ, tt)

            def proj_chunk(dst, w_sb, x_ap, tok0, tt):
                _proj(dst, w_sb, load_chunk(x_ap, tok0, tt), tt)

            # --- emission schedule: batch 0 starts attention right after its
            # first k/q chunks; the rest of its own qkv work is interleaved
            # into the qt=0 window via pre-hooks (scores for chunk c's
            # k-tiles must be emitted after chunk c's projection to keep the
            # in-order PE queue deadlock-free), so the startup is gated by
            # ~2.75MB of DMA spread over the three dynamic queues. ---
            kvx = {
                0: load_chunk(
                    xkvt, 0, 0, engs=(nc.sync, nc.gpsimd, nc.scalar, nc.gpsimd),
                    fine=True,
                )
            }
            q0x = load_chunk(
                xqt, 0, 0, engs=(nc.sync, nc.gpsimd, nc.scalar, nc.sync), fine=True
            )
            # wv must land before v0-proj (~14.5us in); wout isn't needed
            # until ~40us
            wv_sb = persist.tile([128, KO, HD], BF16, tag="wv")
            nc.gpsimd.dma_start(wv_sb[:], wv_d.ap().rearrange("(ko p) m -> p ko m", p=128))
            # prefetch the exp table while the input DMAs stream (first exp
            # fires ~14.5us in)
            dummy = persist.tile([1, 8], F32, tag="dummy")
            nc.vector.memset(dummy[:], 0.0)
            nc.scalar.activation(dummy[:], dummy[:], Exp)
            wout_sb = persist.tile([HD, DIM], BF16, tag="wout")
            nc.scalar.dma_start(wout_sb[:], wout_d.ap())
            _proj(kt_sb[0], wk_sb, kvx[0], 0, scale=SCH_C1, fill_at={4: 2, 6: 3})
            filler(2)
            _proj(qt_sb[0], wq_sb, q0x, 0, fill_at={3: 2, 5: 2, 6: 8, 7: 3})
            filler(6)

            qlx = {}

            def kv_load0(tt):
                return lambda: kvx.__setitem__(tt, load_chunk(xkvt, 0, tt))

            def q_load0(tt):
                return lambda: qlx.__setitem__(tt, load_chunk(xqt, 0, tt))

            def k_proj0(tt):
                return lambda: _proj(kt_sb[0], wk_sb, kvx[tt], tt, scale=SCH_C1)

            def v_proj0(tt):
                def go():
                    _proj(vt_sb[0], wv_sb, kvx.pop(tt), tt)
                    vnat_group(0, tt)

                return go

            def q_proj0(tt):
                return lambda: _proj(qt_sb[0], wq_sb, qlx.pop(tt), tt)

            pre0 = {
                0: [kv_load0(1)],
                1: [v_proj0(0)],
                2: [kv_load0(2), k_proj0(1)],
                3: [kv_load0(3)],
                4: [v_proj0(1)],
                6: [k_proj0(2)],
                8: [q_load0(1), v_proj0(2)],
                10: [q_load0(2), k_proj0(3)],
                12: [q_load0(3), v_proj0(3)],
                16: [q_proj0(1)],
                32: [q_proj0(2)],
                48: [q_proj0(3)],
            }

            nxt = qkv_pieces(1)

            def emit_next():
                p = next(nxt, None)
                if p is not None:
                    p()

            def emit_n(n):
                def go():
                    for _ in range(n):
                        emit_next()

                return go

            def add_flush(hooks, b, qt, at):
                """Norm at (at,1), then the quarter's 4 outproj tiles
                staggered at j=3/7/11/15 so their psum-drain copies spread
                over the whole window instead of bunching ahead of the
                exps in the ACT/DVE FIFOs."""
                hooks.setdefault((at, 1), []).append(lambda: norm_flush(b, qt))
                for i in range(4):
                    hooks.setdefault((at, 3 + 4 * i), []).append(
                        lambda i=i: outproj(
                            b, qt * 4 + i, qt * 4 + i + 1, split_copy=True
                        )
                    )

            hooks0 = {
                (1, 15): [emit_n(4)],
                (2, 15): [emit_n(4)],
                (3, 15): [emit_n(4)],
            }
            add_flush(hooks0, 0, 0, at=1)
            add_flush(hooks0, 0, 1, at=2)
            add_flush(hooks0, 0, 2, at=3)
            attention(0, hooks0, pre0)

            # batch-0's last outproj quarter is held back and dripped into
            # the tail of batch-1's attention (steps (3,6)..(3,15) and the
            # software-pipeline drain) so the PE never idles long enough
            # there for the HAM clock gate to re-throttle it to 1.2GHz
            hooks1 = {
                (0, 0): [lambda: norm_flush(0, 3)],
                (0, 3): [emit_next],
                (0, 8): [emit_next],
                (0, 13): [emit_next],
                (3, 6): [lambda: outproj(0, 12, 13, split_copy=True)],
                (3, 9): [lambda: outproj(0, 13, 14, split_copy=True)],
                (3, 12): [lambda: outproj(0, 14, 15, split_copy=True)],
                (3, 13): [lambda: fillerm(1)],
                (3, 14): [lambda: fillerm(1)],
                (3, 15): [lambda: outproj(0, 15, 16, split_copy=True)],
            }
            add_flush(hooks1, 1, 0, at=1)
            add_flush(hooks1, 1, 1, at=2)
            add_flush(hooks1, 1, 2, at=3)
            attention(1, hooks1)
            # tail: final norm, then the last outproj quarter with both
            # halves of each tile in one (now free) spsum group -- one copy
            # and one DMA per tile -- plus keep-warm fillers so the HAM
            # clock gate doesn't halve the PE clock during the serial
            # norm -> proj -> copy -> DMA chain
            filler(2)
            norm_flush(1, 3)
            filler(2)
            for tt in range(12, 16):
                t_sl = slice(tt * 128, (tt + 1) * 128)
                fps = spsum.tile([128, HPC, QTILE], F32, tag="s", name="projf")
                for nt in range(2):
                    nc.tensor.matmul(
                        fps[:, nt, :],
                        outT[1][:, t_sl],
                        wout_sb[:, nt * 512 : (nt + 1) * 512],
                        start=True,
                        stop=True,
                    )
                fob = ost.tile([128, 2, 512], BF16, tag="o")
                if tt < 14:
                    (nc.scalar.copy if tt % 2 == 0 else nc.vector.tensor_copy)(
                        fob[:], fps[:]
                    )
                    (nc.gpsimd if tt % 2 else nc.sync).dma_start(
                        out_d.ap()[
                            SQ + tt * 128 : SQ + (tt + 1) * 128, :
                        ].rearrange("t (n c) -> t n c", n=2),
                        fob[:],
                    )
                else:
                    # last two tiles: split each copy and DMA across both
                    # engines / both queues so the final drain is parallel
                    nc.scalar.copy(fob[:, 0, :], fps[:, 0, :])
                    nc.vector.tensor_copy(fob[:, 1, :], fps[:, 1, :])
                    for nt in range(2):
                        (nc.sync if nt == 0 else nc.gpsimd).dma_start(
                            out_d.ap()[
                                SQ + tt * 128 : SQ + (tt + 1) * 128,
                                nt * 512 : (nt + 1) * 512,
                            ],
                            fob[:, nt, :],
                        )

    nc.compile()
    return nc


def make_in_maps(x_q, x_kv, W_qkv, W_out):
    x_q = np.asarray(x_q, dtype=np.float32)
    x_kv = np.asarray(x_kv, dtype=np.float32)
    W_qkv = np.asarray(W_qkv, dtype=np.float32)
    W_out = np.asarray(W_out, dtype=np.float32)

    def chunk_tile(x):
        # [TOK, DIM] -> [n_chunks, 128, KO, PCHUNK] with D = ko*128 + p
        xt = x.reshape(TOK, DIM).T.reshape(KO, 128, TOK // PCHUNK, PCHUNK)
        return np.ascontiguousarray(xt.transpose(2, 1, 0, 3)).astype(BF)

    xqt = chunk_tile(x_q)
    xkvt = chunk_tile(x_kv)

    in_maps = []
    for c in range(N_CORES):
        cs = slice(c * HD, (c + 1) * HD)
        in_maps.append(
            {
                "xqt": xqt,
                "xkvt": xkvt,
                "wq": np.ascontiguousarray(W_qkv[:, cs]).astype(BF),
                "wk": np.ascontiguousarray(W_qkv[:, 1024:][:, cs]).astype(BF),
                "wv": np.ascontiguousarray(W_qkv[:, 2048:][:, cs]).astype(BF),
                "wout": np.ascontiguousarray(W_out[cs, :]).astype(BF),
            }
        )
    return in_maps


def combine(partials, b_out):
    """Sum the 8 per-core partial projections and add the bias."""
    acc = np.zeros((TOK, DIM), dtype=np.float32)
    for p in partials:
        acc += np.asarray(p, dtype=np.float32)
    acc += np.asarray(b_out, dtype=np.float32)
    return acc.reshape(B, SQ, DIM)


_STATE = {}


def _get_nc():
    if "nc" not in _STATE:
        _STATE["nc"] = build()
    return _STATE["nc"]


def run(x_q, x_kv, W_qkv, W_out, b_out, trace=False):
    nc = _get_nc()
    in_maps = make_in_maps(x_q, x_kv, W_qkv, W_out)
    res = run_bass_kernel_spmd(nc, in_maps, list(range(N_CORES)), trace=trace)
    out = combine([r["out"] for r in res.results], b_out)
    return out, res


def kernel(x_q, x_kv, W_qkv, W_out, b_out):
    out, _ = run(x_q, x_kv, W_qkv, W_out, b_out, trace=False)
    return out

